# revision 3
# baseline (speedup 1.0000x reference)
"""ChebConv (K=3) Trainium2 kernel, 8-core SPMD.

Math: with lam = lambda_max, c1=-2/lam, c2=2/lam-1, d1=-4/lam, d2=4/lam-2 and
A = D^-1/2 A D^-1/2 (in-degree norm, clamped), the reference output is

    out = feat @ M0 + g @ M1 + q @ M2 + bias,   g = A feat, q = A g
    M0 = W0^T + c2 W1^T + (d2 c2 - 1) W2^T
    M1 = c1 W1^T + (d1 c2 + d2 c1) W2^T
    M2 = d1 c1 W2^T

Device strategy (one NEFF, SPMD on 8 cores):
  - each core receives ONLY its 12544-node feat shard (fp16); a device-side
    AllGather materializes the full padded feat table in HBM (avoids
    replicating 25.7MB x 8 over the host link).
  - dst nodes block-partitioned (128/block, 98 blocks per core). Edges are
    bucketed by (dst block, src chunk) on host; each bucket padded to a
    multiple of 128 "edge tiles".
  - per gather call (16 edge tiles): dma_gather 2048 source rows (fp16,
    256B each) from HBM; build all 16 weighted one-hot [128e x 128dst]
    tiles with TWO batched DVE ops (broadcast is_equal + broadcast mult)
    instead of one tensor_scalar per tile; per tile one matmul
    lhsT=X_tile rhs=onehot accumulating g^T block [128f x 128dst] in PSUM.
  - g blocks are transposed back to node-major via an identity matmul and
    written to a DRAM bounce buffer; one fp16 AllGather shares g across
    cores; hop 2 repeats the same structure on g to get q.
  - dense epilogue per block on TensorE with host-folded M0/M1/M2 + bias;
    fp16 output, upcast on host.
"""
import os
import sys

sys.path.insert(0, "/opt/trn_rl_repo")

import numpy as np

import concourse.bacc as bacc
import concourse.mybir as mybir
import concourse.tile as tile
from concourse import bass_utils

NCORE = 8
BLK = 128
D = 128
NCHUNK = 4
CALL_TILES = 16                      # edge tiles per dma_gather call
CALL_IDX = CALL_TILES * BLK


def _prep(feat, W, bias, lambda_max, src, dst):
    """Host-side graph preprocessing. Returns per-core in_maps + plan."""
    N = feat.shape[0]
    E = src.shape[0]
    src = np.asarray(src).astype(np.int64)
    dst = np.asarray(dst).astype(np.int64)
    feat = np.asarray(feat).astype(np.float32)
    W = np.asarray(W).astype(np.float32)
    bias = np.asarray(bias).astype(np.float32)
    lam = float(np.asarray(lambda_max).reshape(-1)[0])

    npad_unit = NCORE * BLK
    NPAD = ((N + npad_unit - 1) // npad_unit) * npad_unit
    NBLK = NPAD // BLK
    BPC = NBLK // NCORE
    NPC = BPC * BLK
    CHUNK = NPAD // NCHUNK
    assert CHUNK % BLK == 0 and CHUNK < 32767, (NPAD, CHUNK)

    # normalization
    deg = np.bincount(dst, minlength=N).astype(np.float32)
    norm = np.clip(deg, 1.0, None) ** -0.5
    w_all = (norm[src] * norm[dst]).astype(np.float32)

    blk_all = dst // BLK                      # global dst block
    chunk_all = src // CHUNK
    key = (blk_all * NCHUNK + chunk_all).astype(np.int64)
    order = np.argsort(key, kind="stable")
    sk = key[order]

    cnt_flat = np.bincount(key, minlength=NBLK * NCHUNK)
    cnt = cnt_flat.reshape(NCORE, BPC, NCHUNK)
    # tiles per (block-within-core, chunk): max over cores -> shared program
    T = -(-cnt.max(axis=0) // BLK)            # [BPC, NCHUNK]
    # every block needs at least one tile so its PSUM group gets start/stop
    none_mask = T.sum(axis=1) == 0
    T[none_mask, 0] = 1
    tile_off = np.zeros((BPC, NCHUNK), np.int64)
    NT = np.zeros(NCHUNK, np.int64)
    for c in range(NCHUNK):
        tile_off[:, c] = np.cumsum(T[:, c]) - T[:, c]
        NT[c] = T[:, c].sum()
    # pad NT to a multiple of CALL_TILES so call slicing stays in bounds
    NTP = np.array([-(-int(NT[c]) // CALL_TILES) * CALL_TILES for c in range(NCHUNK)])

    # slot position of every edge inside its core's per-chunk stream
    group_starts = np.zeros(NBLK * NCHUNK + 1, np.int64)
    group_starts[1:] = np.cumsum(cnt_flat)
    rank = np.arange(E, dtype=np.int64) - group_starts[sk]
    bb_s = (sk // NCHUNK) % BPC
    core_s = (sk // NCHUNK) // BPC
    c_s = sk % NCHUNK
    pos = tile_off[bb_s, c_s] * BLK + rank

    idx16_all = (src - chunk_all * CHUNK).astype(np.int16)[order]
    w_s = w_all[order]
    dl_s = (dst % BLK).astype(np.float16)[order]

    idxs = [np.zeros((NCORE, NTP[c] * BLK), np.int16) for c in range(NCHUNK)]
    ws = [np.zeros((NCORE, NTP[c] * BLK), np.float16) for c in range(NCHUNK)]
    dls = [np.zeros((NCORE, NTP[c] * BLK), np.float16) for c in range(NCHUNK)]
    for c in range(NCHUNK):
        m = c_s == c
        idxs[c][core_s[m], pos[m]] = idx16_all[m]
        ws[c][core_s[m], pos[m]] = w_s[m]
        dls[c][core_s[m], pos[m]] = dl_s[m]

    # folded dense matrices
    c1 = -2.0 / lam
    c2 = 2.0 / lam - 1.0
    d1 = -4.0 / lam
    d2 = 4.0 / lam - 2.0
    W0T, W1T, W2T = W[0].T, W[1].T, W[2].T
    M0 = W0T + c2 * W1T + (d2 * c2 - 1.0) * W2T
    M1 = c1 * W1T + (d1 * c2 + d2 * c1) * W2T
    M2 = (d1 * c1) * W2T

    featH = np.zeros((NPAD, D), np.float16)
    featH[:N] = feat.astype(np.float16)

    shared = {
        "M0": M0.astype(np.float16),
        "M1": M1.astype(np.float16),
        "M2": M2.astype(np.float16),
        "bias_rep": np.tile(bias[None, :].astype(np.float32), (BLK, 1)),
        "iota": np.tile(np.arange(BLK, dtype=np.float16)[None, :], (BLK, 1)).reshape(
            BLK, 1, BLK
        ),
        "ident": np.eye(BLK, dtype=np.float16),
    }
    in_maps = []
    for k in range(NCORE):
        m = dict(shared)
        m["featLocal"] = featH[k * NPC : (k + 1) * NPC]
        for c in range(NCHUNK):
            m[f"idx{c}"] = np.ascontiguousarray(
                np.tile(idxs[c][k].reshape(-1, 16).T, (8, 1))
            )
            m[f"w{c}"] = np.ascontiguousarray(
                ws[c][k].reshape(-1, BLK).T
            ).reshape(BLK, NTP[c], 1)
            m[f"dl{c}"] = np.ascontiguousarray(
                dls[c][k].reshape(-1, BLK).T
            ).reshape(BLK, NTP[c], 1)
        in_maps.append(m)

    plan = dict(N=N, NPAD=NPAD, BPC=BPC, NPC=NPC, CHUNK=CHUNK,
                T=T, tile_off=tile_off, NT=NT, NTP=NTP)
    return in_maps, plan


def _build(plan):
    """Emit the Bass/Tile program for the shared SPMD NEFF."""
    BPC, NPC, NPAD, CHUNK = plan["BPC"], plan["NPC"], plan["NPAD"], plan["CHUNK"]
    T, tile_off, NT, NTP = plan["T"], plan["tile_off"], plan["NT"], plan["NTP"]
    f16, f32, i16 = mybir.dt.float16, mybir.dt.float32, mybir.dt.int16

    nc = bacc.Bacc("TRN2", target_bir_lowering=False, debug=False,
                   num_devices=NCORE, num_swdge_queues=4,
                   dynamic_dma_scratch_size=32768)
    featL_d = nc.dram_tensor("featLocal", [NPC, D], f16, kind="ExternalInput")
    idx_d = [nc.dram_tensor(f"idx{c}", [128, NTP[c] * 8], i16, kind="ExternalInput")
             for c in range(NCHUNK)]
    w_d = [nc.dram_tensor(f"w{c}", [128, NTP[c], 1], f16, kind="ExternalInput")
           for c in range(NCHUNK)]
    dl_d = [nc.dram_tensor(f"dl{c}", [128, NTP[c], 1], f16, kind="ExternalInput")
            for c in range(NCHUNK)]
    M_d = [nc.dram_tensor(f"M{i}", [D, D], f16, kind="ExternalInput")
           for i in range(3)]
    bias_d = nc.dram_tensor("bias_rep", [BLK, D], f32, kind="ExternalInput")
    iota_d = nc.dram_tensor("iota", [BLK, 1, BLK], f16, kind="ExternalInput")
    ident_d = nc.dram_tensor("ident", [BLK, BLK], f16, kind="ExternalInput")
    out_d = nc.dram_tensor("out", [NPC, D], f16, kind="ExternalOutput")

    with tile.TileContext(nc) as tc:
        with (
            tc.tile_pool(name="const", bufs=1) as cpool,
            tc.tile_pool(name="resident", bufs=1) as rpool,
            tc.tile_pool(name="idxp", bufs=6) as idxpool,
            tc.tile_pool(name="streams", bufs=2) as spool,
            tc.tile_pool(name="ow", bufs=2) as owpool,
            tc.tile_pool(name="small", bufs=3) as npool,
            tc.tile_pool(name="psum", bufs=2, space="PSUM") as psum,
            tc.tile_pool(name="dram", bufs=1, space="DRAM") as dram,
        ):
            iota_t = cpool.tile([BLK, 1, BLK], f16)
            nc.sync.dma_start(out=iota_t[:], in_=iota_d[:])
            ident_t = cpool.tile([BLK, BLK], f16)
            nc.sync.dma_start(out=ident_t[:], in_=ident_d[:])
            M_t = []
            for i in range(3):
                t = cpool.tile([D, D], f16, tag=f"M{i}")
                nc.sync.dma_start(out=t[:], in_=M_d[i][:])
                M_t.append(t)
            bias_t = cpool.tile([BLK, D], f32)
            nc.sync.dma_start(out=bias_t[:], in_=bias_d[:])
            w_t, dl_t = [], []
            for c in range(NCHUNK):
                wt = rpool.tile([128, NTP[c], 1], f16, tag=f"w{c}")
                nc.sync.dma_start(out=wt[:], in_=w_d[c][:])
                w_t.append(wt)
                dt_ = rpool.tile([128, NTP[c], 1], f16, tag=f"dl{c}")
                nc.sync.dma_start(out=dt_[:], in_=dl_d[c][:])
                dl_t.append(dt_)
            featT = rpool.tile([128, NPC], f16, tag="featT")
            nc.sync.dma_start_transpose(out=featT[:], in_=featL_d[:])
            gT = rpool.tile([128, NPC], f16, tag="gT")

            # device-side replication of feat: 3.2MB shard -> 25.7MB table.
            # (collectives cannot read IO tensors, so bounce through DRAM)
            featL_b = dram.tile([NPC, D], f16)
            nc.sync.dma_start(out=featL_b[:], in_=featL_d[:])
            featH_dev = dram.tile([NPAD, D], f16)
            nc.gpsimd.collective_compute(
                "AllGather",
                mybir.AluOpType.bypass,
                ins=[featL_b[:].opt()],
                outs=[featH_dev[:].opt()],
                replica_groups=[list(range(NCORE))],
            )

            cc_in = dram.tile([NPC, D], f16)
            cc_out = dram.tile([NPAD, D], f16)

            def run_hop(src_views, out_hook):
                emitted = [0] * NCHUNK
                bufs = {}
                ows = {}

                def ensure_call(c, j):
                    while emitted[c] <= j:
                        jj = emitted[c]
                        n_t = min(CALL_TILES, int(NT[c]) - jj * CALL_TILES)
                        n_idx = n_t * BLK
                        ib = idxpool.tile([128, CALL_IDX // 16], i16, tag="idx")
                        nc.sync.dma_start(
                            out=ib[:, : n_idx // 16],
                            in_=idx_d[c][:, jj * (CALL_IDX // 16):
                                         jj * (CALL_IDX // 16) + n_idx // 16],
                        )
                        buf = spool.tile([128, CALL_TILES, BLK], f16, tag=f"s{c}")
                        nc.gpsimd.dma_gather(
                            out_ap=buf[:, :n_t, :],
                            in_ap=src_views[c],
                            idxs_ap=ib[:, : n_idx // 16],
                            num_idxs=n_idx,
                            num_idxs_reg=n_idx,
                            elem_size=D,
                            single_packet=False,
                            queue_num=c,
                        )
                        bufs[(c, jj)] = buf
                        # batched one-hot build for all tiles of this call:
                        # ow[p,t,l] = (iota[p,0,l] == dl[p,jj*CT+t,0]) * w[...]
                        ow = owpool.tile([128, CALL_TILES, BLK], f16, tag=f"ow{c}")
                        sl = slice(jj * CALL_TILES, (jj + 1) * CALL_TILES)
                        nc.vector.tensor_tensor(
                            out=ow[:],
                            in0=iota_t[:].to_broadcast([128, CALL_TILES, BLK]),
                            in1=dl_t[c][:, sl, :].to_broadcast(
                                [128, CALL_TILES, BLK]),
                            op=mybir.AluOpType.is_equal,
                        )
                        nc.vector.tensor_tensor(
                            out=ow[:],
                            in0=ow[:],
                            in1=w_t[c][:, sl, :].to_broadcast(
                                [128, CALL_TILES, BLK]),
                            op=mybir.AluOpType.mult,
                        )
                        ows[(c, jj)] = ow
                        emitted[c] += 1

                for bb in range(BPC):
                    total = int(T[bb].sum())
                    acc = psum.tile([128, BLK], f32, tag="acc", space="PSUM")
                    done = 0
                    for c in range(NCHUNK):
                        for t in range(int(T[bb][c])):
                            p = int(tile_off[bb][c]) + t
                            j, slot = divmod(p, CALL_TILES)
                            ensure_call(c, j)
                            buf = bufs[(c, j)]
                            ow = ows[(c, j)]
                            done += 1
                            nc.tensor.matmul(
                                out=acc[:],
                                lhsT=buf[:, slot, :],
                                rhs=ow[:, slot, :],
                                start=(done == 1),
                                stop=(done == total),
                            )
                    out_hook(bb, acc)

            # ---- hop 1: g = A feat ----
            def hop1_out(bb, acc):
                sl = slice(bb * BLK, (bb + 1) * BLK)
                nc.vector.tensor_copy(out=gT[:, sl], in_=acc[:])
                tp = psum.tile([128, BLK], f32, tag="tp", space="PSUM")
                nc.tensor.matmul(out=tp[:], lhsT=gT[:, sl], rhs=ident_t[:],
                                 start=True, stop=True)
                gn = npool.tile([BLK, D], f16, tag="gn")
                nc.vector.tensor_copy(out=gn[:], in_=tp[:])
                nc.sync.dma_start(out=cc_in[sl, :], in_=gn[:])

            # ---- hop 2: q = A g, fused epilogue ----
            def hop2_out(bb, acc):
                sl = slice(bb * BLK, (bb + 1) * BLK)
                qT_t = npool.tile([128, BLK], f16, tag="qT")
                nc.vector.tensor_copy(out=qT_t[:], in_=acc[:])
                out_ps = psum.tile([128, BLK], f32, tag="outp", space="PSUM")
                nc.tensor.matmul(out=out_ps[:], lhsT=featT[:, sl], rhs=M_t[0][:],
                                 start=True, stop=False)
                nc.tensor.matmul(out=out_ps[:], lhsT=gT[:, sl], rhs=M_t[1][:],
                                 start=False, stop=False)
                nc.tensor.matmul(out=out_ps[:], lhsT=qT_t[:], rhs=M_t[2][:],
                                 start=False, stop=True)
                ob = npool.tile([BLK, D], f16, tag="ob")
                nc.vector.tensor_tensor(out=ob[:], in0=out_ps[:], in1=bias_t[:],
                                        op=mybir.AluOpType.add)
                nc.sync.dma_start(out=out_d[sl, :], in_=ob[:])

            run_hop(
                [featH_dev[c * CHUNK : (c + 1) * CHUNK, :] for c in range(NCHUNK)],
                hop1_out,
            )
            nc.gpsimd.collective_compute(
                "AllGather",
                mybir.AluOpType.bypass,
                ins=[cc_in.opt()],
                outs=[cc_out.opt()],
                replica_groups=[list(range(NCORE))],
            )
            run_hop(
                [cc_out[c * CHUNK : (c + 1) * CHUNK, :] for c in range(NCHUNK)],
                hop2_out,
            )

    nc.compile()
    return nc


def kernel(feat, W, bias, lambda_max, src, dst):
    in_maps, plan = _prep(feat, W, bias, lambda_max, src, dst)
    nc = _build(plan)
    res = bass_utils.run_bass_kernel_spmd(nc, in_maps, core_ids=list(range(NCORE)))
    # stashed for external benchmarking harnesses (not used by the kernel)
    kernel.last_nc = nc
    kernel.last_in_maps = in_maps
    kernel.last_plan = plan
    out = np.concatenate([res.results[k]["out"] for k in range(NCORE)], axis=0)
    return np.ascontiguousarray(out[: plan["N"]]).astype(np.float32)


# revision 5
# speedup vs baseline: 1.0879x; 1.0879x over previous
"""ChebConv (K=3) Trainium2 kernel, 8-core SPMD.

Math: with lam = lambda_max, c1=-2/lam, c2=2/lam-1, d1=-4/lam, d2=4/lam-2 and
A = D^-1/2 A D^-1/2 (in-degree norm, clamped), the reference output is

    out = feat @ M0 + g @ M1 + q @ M2 + bias,   g = A feat, q = A g
    M0 = W0^T + c2 W1^T + (d2 c2 - 1) W2^T
    M1 = c1 W1^T + (d1 c2 + d2 c1) W2^T
    M2 = d1 c1 W2^T

Norm folding: feat' = norm (.) feat is prepared on host; each SpMM hop
computes the unnormalized segment sum S = sum_e feat'[src_e] per dst and the
dst-side norm is applied as a per-partition scale after the transpose
(hop 1) or in the epilogue (hop 2). Edge padding slots carry dst-lane 255,
which the is_equal one-hot build maps to an all-zero column.

Device strategy (one NEFF, SPMD on 8 cores):
  - each core receives ONLY its 12544-node feat shard (fp16); a device-side
    AllGather materializes the full padded feat table in HBM.
  - dst nodes block-partitioned (128/block, 98 blocks per core). Edges are
    bucketed by (dst block, src chunk) on host; buckets padded to whole
    128-edge tiles.
  - per gather call (16 edge tiles): dma_gather 2048 source rows (fp16,
    256B each) from HBM; ONE batched DVE is_equal builds all 16 one-hot
    [128e x 128dst] tiles; per tile one matmul lhsT=X_tile rhs=onehot
    accumulates S^T for the dst block in PSUM.
  - g blocks are transposed to node-major via an identity matmul, scaled by
    norm[dst], and written to a DRAM bounce buffer; one fp16 AllGather
    shares g across cores; hop 2 repeats the structure on g to get S2.
  - dense epilogue per block on TensorE: psA = feat@M0 + bias (K=1 ones
    matmul), psB = S1@M1 + S2@M2, out = norm (.) psB + psA; fp16 output,
    upcast on host.
"""
import os
import sys

sys.path.insert(0, "/opt/trn_rl_repo")

import numpy as np

import concourse.bacc as bacc
import concourse.mybir as mybir
import concourse.tile as tile
from concourse import bass_utils

NCORE = 8
BLK = 128
D = 128
NCHUNK = 4
CALL_TILES = 16                      # edge tiles per dma_gather call
CALL_IDX = CALL_TILES * BLK


def _prep(feat, W, bias, lambda_max, src, dst):
    """Host-side graph preprocessing. Returns per-core in_maps + plan."""
    N = feat.shape[0]
    E = src.shape[0]
    src = np.asarray(src).astype(np.int64)
    dst = np.asarray(dst).astype(np.int64)
    feat = np.asarray(feat).astype(np.float32)
    W = np.asarray(W).astype(np.float32)
    bias = np.asarray(bias).astype(np.float32)
    lam = float(np.asarray(lambda_max).reshape(-1)[0])

    npad_unit = NCORE * BLK
    NPAD = ((N + npad_unit - 1) // npad_unit) * npad_unit
    NBLK = NPAD // BLK
    BPC = NBLK // NCORE
    NPC = BPC * BLK
    CHUNK = NPAD // NCHUNK
    assert CHUNK % BLK == 0 and CHUNK < 32767, (NPAD, CHUNK)

    # normalization
    deg = np.bincount(dst, minlength=N).astype(np.float32)
    norm = np.clip(deg, 1.0, None) ** -0.5
    norm_full = np.ones(NPAD, np.float32)
    norm_full[:N] = norm

    blk_all = dst // BLK                      # global dst block
    chunk_all = src // CHUNK
    key = (blk_all * NCHUNK + chunk_all).astype(np.int64)
    order = np.argsort(key, kind="stable")
    sk = key[order]

    cnt_flat = np.bincount(key, minlength=NBLK * NCHUNK)
    cnt = cnt_flat.reshape(NCORE, BPC, NCHUNK)
    # tiles per (block-within-core, chunk): max over cores -> shared program
    T = -(-cnt.max(axis=0) // BLK)            # [BPC, NCHUNK]
    # every block needs at least one tile so its PSUM group gets start/stop
    none_mask = T.sum(axis=1) == 0
    T[none_mask, 0] = 1
    tile_off = np.zeros((BPC, NCHUNK), np.int64)
    NT = np.zeros(NCHUNK, np.int64)
    for c in range(NCHUNK):
        tile_off[:, c] = np.cumsum(T[:, c]) - T[:, c]
        NT[c] = T[:, c].sum()
    # pad NT to a multiple of CALL_TILES so call slicing stays in bounds
    NTP = np.array([-(-int(NT[c]) // CALL_TILES) * CALL_TILES for c in range(NCHUNK)])

    # slot position of every edge inside its core's per-chunk stream
    group_starts = np.zeros(NBLK * NCHUNK + 1, np.int64)
    group_starts[1:] = np.cumsum(cnt_flat)
    rank = np.arange(E, dtype=np.int64) - group_starts[sk]
    bb_s = (sk // NCHUNK) % BPC
    core_s = (sk // NCHUNK) // BPC
    c_s = sk % NCHUNK
    pos = tile_off[bb_s, c_s] * BLK + rank

    idx16_all = (src - chunk_all * CHUNK).astype(np.int16)[order]
    dl_s = (dst % BLK).astype(np.float16)[order]

    idxs = [np.zeros((NCORE, NTP[c] * BLK), np.int16) for c in range(NCHUNK)]
    # padding slots carry lane 255 -> is_equal against iota (0..127) gives 0
    dls = [np.full((NCORE, NTP[c] * BLK), 255.0, np.float16) for c in range(NCHUNK)]
    for c in range(NCHUNK):
        m = c_s == c
        idxs[c][core_s[m], pos[m]] = idx16_all[m]
        dls[c][core_s[m], pos[m]] = dl_s[m]

    # folded dense matrices
    c1 = -2.0 / lam
    c2 = 2.0 / lam - 1.0
    d1 = -4.0 / lam
    d2 = 4.0 / lam - 2.0
    W0T, W1T, W2T = W[0].T, W[1].T, W[2].T
    M0 = W0T + c2 * W1T + (d2 * c2 - 1.0) * W2T
    M1 = c1 * W1T + (d1 * c2 + d2 * c1) * W2T
    M2 = (d1 * c1) * W2T

    # norm-folded features (src-side scale), fp16, zero padded
    featH = np.zeros((NPAD, D), np.float16)
    featH[:N] = (feat * norm[:, None]).astype(np.float16)

    iota_rep = np.tile(np.arange(BLK, dtype=np.float16), (BLK, CALL_TILES))

    shared = {
        "M0": M0.astype(np.float16),
        "M1": M1.astype(np.float16),
        "M2": M2.astype(np.float16),
        "bias16": bias.astype(np.float16).reshape(1, D),
        "iota_rep": iota_rep.reshape(BLK, CALL_TILES, BLK),
        "ident": np.eye(BLK, dtype=np.float16),
    }
    featR = np.zeros((NPAD, D), np.float16)
    featR[:N] = feat.astype(np.float16)

    in_maps = []
    for k in range(NCORE):
        m = dict(shared)
        m["featLocal"] = featH[k * NPC : (k + 1) * NPC]
        m["featRaw"] = featR[k * NPC : (k + 1) * NPC]
        nrm_k = norm_full[k * NPC : (k + 1) * NPC].reshape(BPC, BLK).T
        m["nrm"] = np.ascontiguousarray(nrm_k)
        m["nrm2"] = np.ascontiguousarray(nrm_k * nrm_k)
        for c in range(NCHUNK):
            m[f"idx{c}"] = np.ascontiguousarray(
                np.tile(idxs[c][k].reshape(-1, 16).T, (8, 1))
            )
            m[f"dl{c}"] = np.ascontiguousarray(
                dls[c][k].reshape(-1, BLK).T
            ).reshape(BLK, NTP[c], 1)
        in_maps.append(m)

    plan = dict(N=N, NPAD=NPAD, BPC=BPC, NPC=NPC, CHUNK=CHUNK,
                T=T, tile_off=tile_off, NT=NT, NTP=NTP)
    return in_maps, plan


def _build(plan):
    """Emit the Bass/Tile program for the shared SPMD NEFF."""
    BPC, NPC, NPAD, CHUNK = plan["BPC"], plan["NPC"], plan["NPAD"], plan["CHUNK"]
    T, tile_off, NT, NTP = plan["T"], plan["tile_off"], plan["NT"], plan["NTP"]
    f16, f32, i16 = mybir.dt.float16, mybir.dt.float32, mybir.dt.int16

    nc = bacc.Bacc("TRN2", target_bir_lowering=False, debug=False,
                   num_devices=NCORE, num_swdge_queues=4,
                   dynamic_dma_scratch_size=32768)
    featL_d = nc.dram_tensor("featLocal", [NPC, D], f16, kind="ExternalInput")
    idx_d = [nc.dram_tensor(f"idx{c}", [128, NTP[c] * 8], i16, kind="ExternalInput")
             for c in range(NCHUNK)]
    dl_d = [nc.dram_tensor(f"dl{c}", [128, NTP[c], 1], f16, kind="ExternalInput")
            for c in range(NCHUNK)]
    M_d = [nc.dram_tensor(f"M{i}", [D, D], f16, kind="ExternalInput")
           for i in range(3)]
    bias_d = nc.dram_tensor("bias16", [1, D], f16, kind="ExternalInput")
    iota_d = nc.dram_tensor("iota_rep", [BLK, CALL_TILES, BLK], f16,
                            kind="ExternalInput")
    ident_d = nc.dram_tensor("ident", [BLK, BLK], f16, kind="ExternalInput")
    nrm_d = nc.dram_tensor("nrm", [BLK, BPC], f32, kind="ExternalInput")
    nrm2_d = nc.dram_tensor("nrm2", [BLK, BPC], f32, kind="ExternalInput")
    featR_d = nc.dram_tensor("featRaw", [NPC, D], f16, kind="ExternalInput")
    out_d = nc.dram_tensor("out", [NPC, D], f16, kind="ExternalOutput")

    with tile.TileContext(nc) as tc:
        with (
            tc.tile_pool(name="const", bufs=1) as cpool,
            tc.tile_pool(name="resident", bufs=1) as rpool,
            tc.tile_pool(name="idxp", bufs=6) as idxpool,
            tc.tile_pool(name="streams", bufs=2) as spool,
            tc.tile_pool(name="ow", bufs=2) as owpool,
            tc.tile_pool(name="small", bufs=3) as npool,
            tc.tile_pool(name="psum", bufs=2, space="PSUM") as psum,
            tc.tile_pool(name="dram", bufs=1, space="DRAM") as dram,
        ):
            iota_t = cpool.tile([BLK, CALL_TILES, BLK], f16)
            nc.sync.dma_start(out=iota_t[:], in_=iota_d[:])
            ident_t = cpool.tile([BLK, BLK], f16)
            nc.sync.dma_start(out=ident_t[:], in_=ident_d[:])
            M_t = []
            for i in range(3):
                t = cpool.tile([D, D], f16, tag=f"M{i}")
                nc.sync.dma_start(out=t[:], in_=M_d[i][:])
                M_t.append(t)
            bias_t = cpool.tile([1, D], f16, tag="bias")
            nc.sync.dma_start(out=bias_t[:], in_=bias_d[:])
            ones1 = cpool.tile([1, D], f16, tag="ones1")
            nc.vector.memset(ones1[:], 1.0)
            nrm_t = cpool.tile([BLK, BPC], f32, tag="nrm")
            nc.sync.dma_start(out=nrm_t[:], in_=nrm_d[:])
            nrm2_t = cpool.tile([BLK, BPC], f32, tag="nrm2")
            nc.sync.dma_start(out=nrm2_t[:], in_=nrm2_d[:])
            dl_t = []
            for c in range(NCHUNK):
                dt_ = rpool.tile([128, NTP[c], 1], f16, tag=f"dl{c}")
                nc.sync.dma_start(out=dt_[:], in_=dl_d[c][:])
                dl_t.append(dt_)
            featT = rpool.tile([128, NPC], f16, tag="featT")
            nc.sync.dma_start_transpose(out=featT[:], in_=featR_d[:])
            gT = rpool.tile([128, NPC], f16, tag="gT")

            # device-side replication of feat: 3.2MB shard -> 25.7MB table.
            # (collectives cannot read IO tensors, so bounce through DRAM)
            featL_b = dram.tile([NPC, D], f16)
            nc.sync.dma_start(out=featL_b[:], in_=featL_d[:])
            featH_dev = dram.tile([NPAD, D], f16)
            nc.gpsimd.collective_compute(
                "AllGather",
                mybir.AluOpType.bypass,
                ins=[featL_b[:].opt()],
                outs=[featH_dev[:].opt()],
                replica_groups=[list(range(NCORE))],
            )

            cc_in = dram.tile([NPC, D], f16)
            cc_out = dram.tile([NPAD, D], f16)

            def run_hop(src_views, out_hook):
                emitted = [0] * NCHUNK
                bufs = {}
                ows = {}

                def ensure_call(c, j):
                    while emitted[c] <= j:
                        jj = emitted[c]
                        n_t = min(CALL_TILES, int(NT[c]) - jj * CALL_TILES)
                        n_idx = n_t * BLK
                        ib = idxpool.tile([128, CALL_IDX // 16], i16, tag="idx")
                        nc.sync.dma_start(
                            out=ib[:, : n_idx // 16],
                            in_=idx_d[c][:, jj * (CALL_IDX // 16):
                                         jj * (CALL_IDX // 16) + n_idx // 16],
                        )
                        buf = spool.tile([128, CALL_TILES, BLK], f16, tag=f"s{c}")
                        nc.gpsimd.dma_gather(
                            out_ap=buf[:, :n_t, :],
                            in_ap=src_views[c],
                            idxs_ap=ib[:, : n_idx // 16],
                            num_idxs=n_idx,
                            num_idxs_reg=n_idx,
                            elem_size=D,
                            single_packet=False,
                            queue_num=c,
                        )
                        bufs[(c, jj)] = buf
                        # batched one-hot build for all tiles of this call:
                        # ow[p,t,l] = (iota_rep[p,t,l] == dl[p,jj*CT+t,0])
                        ow = owpool.tile([128, CALL_TILES, BLK], f16, tag=f"ow{c}")
                        sl = slice(jj * CALL_TILES, (jj + 1) * CALL_TILES)
                        nc.vector.tensor_tensor(
                            out=ow[:],
                            in0=iota_t[:],
                            in1=dl_t[c][:, sl, :].to_broadcast(
                                [128, CALL_TILES, BLK]),
                            op=mybir.AluOpType.is_equal,
                        )
                        ows[(c, jj)] = ow
                        emitted[c] += 1

                for bb in range(BPC):
                    total = int(T[bb].sum())
                    acc = psum.tile([128, BLK], f32, tag="acc", space="PSUM")
                    done = 0
                    for c in range(NCHUNK):
                        for t in range(int(T[bb][c])):
                            p = int(tile_off[bb][c]) + t
                            j, slot = divmod(p, CALL_TILES)
                            ensure_call(c, j)
                            buf = bufs[(c, j)]
                            ow = ows[(c, j)]
                            done += 1
                            nc.tensor.matmul(
                                out=acc[:],
                                lhsT=buf[:, slot, :],
                                rhs=ow[:, slot, :],
                                start=(done == 1),
                                stop=(done == total),
                            )
                    out_hook(bb, acc)

            # ---- hop 1: S1 = sum_e feat'[src], g = nrm (.) S1 ----
            def hop1_out(bb, acc):
                sl = slice(bb * BLK, (bb + 1) * BLK)
                nc.vector.tensor_copy(out=gT[:, sl], in_=acc[:])
                tp = psum.tile([128, BLK], f32, tag="tp", space="PSUM")
                nc.tensor.matmul(out=tp[:], lhsT=gT[:, sl], rhs=ident_t[:],
                                 start=True, stop=True)
                gn = npool.tile([BLK, D], f16, tag="gn")
                nc.vector.tensor_scalar(
                    out=gn[:], in0=tp[:], scalar1=nrm2_t[:, bb : bb + 1],
                    scalar2=None, op0=mybir.AluOpType.mult,
                )
                nc.sync.dma_start(out=cc_in[sl, :], in_=gn[:])

            # ---- hop 2: S2 = sum_e g[src], fused epilogue ----
            def hop2_out(bb, acc):
                sl = slice(bb * BLK, (bb + 1) * BLK)
                qT_t = npool.tile([128, BLK], f16, tag="qT")
                nc.vector.tensor_copy(out=qT_t[:], in_=acc[:])
                psA = psum.tile([128, BLK], f32, tag="psA", space="PSUM")
                nc.tensor.matmul(out=psA[:], lhsT=featT[:, sl], rhs=M_t[0][:],
                                 start=True, stop=False)
                nc.tensor.matmul(out=psA[:], lhsT=ones1[:], rhs=bias_t[:],
                                 start=False, stop=True)
                psB = psum.tile([128, BLK], f32, tag="psB", space="PSUM")
                nc.tensor.matmul(out=psB[:], lhsT=gT[:, sl], rhs=M_t[1][:],
                                 start=True, stop=False)
                nc.tensor.matmul(out=psB[:], lhsT=qT_t[:], rhs=M_t[2][:],
                                 start=False, stop=True)
                tsb = npool.tile([BLK, D], f32, tag="tsb")
                nc.vector.tensor_scalar(
                    out=tsb[:], in0=psB[:], scalar1=nrm_t[:, bb : bb + 1],
                    scalar2=None, op0=mybir.AluOpType.mult,
                )
                ob = npool.tile([BLK, D], f16, tag="ob")
                nc.vector.tensor_tensor(out=ob[:], in0=tsb[:], in1=psA[:],
                                        op=mybir.AluOpType.add)
                nc.sync.dma_start(out=out_d[sl, :], in_=ob[:])

            run_hop(
                [featH_dev[c * CHUNK : (c + 1) * CHUNK, :] for c in range(NCHUNK)],
                hop1_out,
            )
            nc.gpsimd.collective_compute(
                "AllGather",
                mybir.AluOpType.bypass,
                ins=[cc_in.opt()],
                outs=[cc_out.opt()],
                replica_groups=[list(range(NCORE))],
            )
            run_hop(
                [cc_out[c * CHUNK : (c + 1) * CHUNK, :] for c in range(NCHUNK)],
                hop2_out,
            )

    nc.compile()
    return nc


def kernel(feat, W, bias, lambda_max, src, dst):
    in_maps, plan = _prep(feat, W, bias, lambda_max, src, dst)
    nc = _build(plan)
    res = bass_utils.run_bass_kernel_spmd(nc, in_maps, core_ids=list(range(NCORE)))
    # stashed for external benchmarking harnesses (not used by the kernel)
    kernel.last_nc = nc
    kernel.last_in_maps = in_maps
    kernel.last_plan = plan
    out = np.concatenate([res.results[k]["out"] for k in range(NCORE)], axis=0)
    return np.ascontiguousarray(out[: plan["N"]]).astype(np.float32)
